# revision 38
# baseline (speedup 1.0000x reference)
"""Linear attention (non-causal, elu+1 feature map) on 8 Trainium2 cores.

Math per (batch b, head h), with phi(x) = elu(x)+1 = min(exp(x),1) + relu(x):
    C_aug = phi(K)^T @ [V | 1]        # (64, 65): context (64x64) + k_sum col
    numer = phi(Q) @ C                # (T, 64)
    denom = phi(Q) @ k_sum            # (T,)
    out   = numer / denom             # eps=1e-6 negligible vs denom ~1e5

Sharding: 16 heads / 8 cores = 2 heads per core, all 4 batches per core.
Everything on device is fp16 (halves HBM traffic; rel err ~1.3e-2 < 2e-2).

Both heads are fused into single 128-wide matmuls:
  mm1: lhsT = [phiK0 | phiK1] (128t x 128), rhs = [VA0 | VA1] (128t x 130)
       -> psum (128 x 130); diagonal 64x65 blocks are C_aug per head,
       accumulated over 32 t-tiles as TWO matmuls per tile (min-part from
       tk, relu-part from kv in place) so phi-K's add never runs (C is
       linear in phiK).
  mm_d: lhsT = phiQ chunk (128e x 128t), rhs = blockdiag ksum (128 x 2)
       -> denom psum (128t x 2) per chunk, all 32 chunks in one bank so
       a single reciprocal_approx_fast covers them.
  mm2: same lhsT, rhs = blockdiag C (128 x 128) -> numer psum; 8 chunks
       fill a 2-bank psum tile (1024 f32), so normalize+evac is one dense
       1024-col DVE multiply per group (psum fp32 forces 1x mode).

All DMA rides the Sync HW descriptor ring (the GpSimd ring measured
consistently slower end-to-end).  phi(K) runs in 1024-col chunks
chasing the kv DMA so mm1 starts early; phi(Q) overlaps mm1.  The last
batch streams each evac group out immediately, final group split in two
512-col pieces so the closing DMA waits on almost nothing.

Engine notes (all measured on HW): GpSimd bulk elementwise is ~20x
slower than DVE and starves DVE via shared SBUF ports - only memsets go
there.  scalar_tensor_tensor has no 2x mode (7.8us/pass), but the
dual-op tensor_scalar DOES run 4x: phi(Q) uses the identity
elu(q)+1 == min(exp q, relu(q)+1) so relu+1 is one 4x pass and the
min one 2x tensor_tensor - two DVE passes instead of three (-4.3us
measured off the pacing engine).  The same identity loses on the K
side: materializing phi(K) adds a 2x min pass per chunk to DVE,
whereas the 2-matmul PE fold keeps DVE at one 4x pass.  The emission order below is empirically
phase-tuned: the ~14us batch period sits near the device's ~13.7us
full/half-speed throttle duty cycle, so seemingly-neutral reorderings
(coarser phi-K chunks, split qt DMA, skewed cross-batch emission)
measured 5-15us slower; do not perturb without re-measuring several
times - run-to-run variance on identical code spans ~80-95us depending
on device thermal state.

Device layouts (per core, all fp16, partition dim first, all APs dense):
    qt: (B, 128, 4096)  qt[b, hh*64+e, n*128+j] = Q[b, t=j*32+n, ch]
    kv: (B, 128, 8256)  cols 0:4096   = K  [n, h, e] (n*128+h*64+e)
                        cols 4096:8256= VA [n, h, m] (n*130+h*65+m, m=64 ones)
                        partition p <-> t = p*32+n
    o:  (B, 128, 4096)  o[b, p, n*128+h*64+e] = out[b, t=p*32+n, h*64+e]
The t = p*32+n tiling gives every DMA 4-8 KB contiguous per partition.
"""

from contextlib import ExitStack

import numpy as np

import concourse.bacc as bacc
import concourse.bass as bass
import concourse.mybir as mybir
import concourse.tile as tile
from concourse.bass_utils import run_bass_kernel_spmd

B = 4
T = 4096
D = 1024
H = 16
E = 64
EA = E + 1
W2 = 2 * EA  # 130 cols: both heads' [VA]
NCORES = 8
HPC = H // NCORES  # 2 heads per core
P = 128
NT = T // P  # 32 t-tiles
KC = HPC * NT * E  # 4096 k-region cols
VC = HPC * NT * EA  # 4160 va-region cols
KV = KC + VC  # 8256
GRP = 8  # mm2 chunks per evac group (8*128 fp32 = 4096 B = 2 psum banks)
NCH = 4  # phi-K chunks per batch
DT = mybir.dt.float16
F32 = mybir.dt.float32
AF = mybir.ActivationFunctionType
ALU = mybir.AluOpType
F16 = np.float16


def build_nc():
    nc = bacc.Bacc("TRN2", target_bir_lowering=False, debug=False)
    qt = nc.dram_tensor("qt", [B, P, T], DT, kind="ExternalInput").ap()
    kv = nc.dram_tensor("kv", [B, P, KV], DT, kind="ExternalInput").ap()
    o = nc.dram_tensor("o", [B, P, T], DT, kind="ExternalOutput").ap()

    with tile.TileContext(nc) as tc, ExitStack() as ctx:
        qt_pool = ctx.enter_context(tc.tile_pool(name="qt", bufs=3))
        kv_pool = ctx.enter_context(tc.tile_pool(name="kv", bufs=3))
        tmpk_pool = ctx.enter_context(tc.tile_pool(name="tmpk", bufs=3))
        tmpq_pool = ctx.enter_context(tc.tile_pool(name="tmpq", bufs=2))
        c_pool = ctx.enter_context(tc.tile_pool(name="c", bufs=2))
        ks_pool = ctx.enter_context(tc.tile_pool(name="ks", bufs=2))
        r_pool = ctx.enter_context(tc.tile_pool(name="r", bufs=2))
        out_pool = ctx.enter_context(tc.tile_pool(name="out", bufs=3))
        psc_pool = ctx.enter_context(tc.tile_pool(name="psc", bufs=2, space="PSUM"))
        pso_pool = ctx.enter_context(tc.tile_pool(name="pso", bufs=2, space="PSUM"))
        psd_pool = ctx.enter_context(tc.tile_pool(name="psd", bufs=2, space="PSUM"))

        for b in range(B):
            kv_t = kv_pool.tile([P, KV], DT)
            qt_t = qt_pool.tile([P, T], DT)
            kq = KC // 4
            if b == 0:
                # batch 0 is the pipeline head. Everything rides the Sync
                # HW ring (the GpSimd ring measured consistently slower);
                # ring FIFO = need order: K chunks feed the exp chain, VA
                # feeds mm1's rhs, qt halves feed the interleaved expQ.
                vh = KC + (NT // 2) * W2
                nc.sync.dma_start(kv_t[:, 0:kq], kv[b, :, 0:kq])
                nc.sync.dma_start(kv_t[:, KC:vh], kv[b, :, KC:vh])
                for i in range(1, 4):
                    nc.sync.dma_start(
                        kv_t[:, i * kq : (i + 1) * kq],
                        kv[b, :, i * kq : (i + 1) * kq],
                    )
                nc.sync.dma_start(kv_t[:, vh:KV], kv[b, :, vh:KV])
                nc.sync.dma_start(qt_t[:], qt[b])
            else:
                nc.sync.dma_start(kv_t[:, 0 : KC // 2], kv[b, :, 0 : KC // 2])
                nc.sync.dma_start(kv_t[:, KC // 2 : KC], kv[b, :, KC // 2 : KC])
                nc.sync.dma_start(kv_t[:, KC:KV], kv[b, :, KC:KV])
                nc.sync.dma_start(qt_t[:], qt[b])

            # phi(K) = min(exp K, 1) + relu(K), but the add never happens:
            # mm1 accumulates BOTH parts into psum (C is linear in phiK).
            # Per tile: 2 matmuls (relu part from kv_t, min part from tk).
            psc = psc_pool.tile([P, 512], F32)  # full psum bank
            tq = tmpq_pool.tile([P, T], DT)
            kchunk = KC // NCH
            tpc = NT // NCH
            for c in range(NCH):
                kreg = kv_t[:, c * kchunk : (c + 1) * kchunk]
                tk = tmpk_pool.tile([P, kchunk], DT)
                nc.scalar.activation(tk[:], kreg, AF.Exp)
                nc.vector.tensor_scalar_min(tk[:], tk[:], 1.0)
                if b == 0:
                    # head batch: DVE is idle here and ACT's serial
                    # exp chunk chain gates the first matmuls
                    nc.vector.tensor_scalar_max(kreg, kreg, 0.0)
                else:
                    nc.scalar.activation(kreg, kreg, AF.Relu)
                for n in range(c * tpc, (c + 1) * tpc):
                    va = kv_t[:, KC + n * W2 : KC + (n + 1) * W2]
                    off = (n - c * tpc) * P
                    nc.tensor.matmul(
                        psc[:, 0:W2],
                        lhsT=kv_t[:, n * P : (n + 1) * P],
                        rhs=va,
                        start=(n == 0),
                        stop=False,
                    )
                    nc.tensor.matmul(
                        psc[:, 0:W2],
                        lhsT=tk[:, off : off + P],
                        rhs=va,
                        start=False,
                        stop=(n == NT - 1),
                    )
            # phi(Q) via the identity elu(q)+1 == min(exp q, relu(q)+1)
            # (for q>0, e^q >= q+1 so min picks q+1; for q<=0 relu+1 = 1
            # clamps exp).  relu(q)+1 is ONE dual-op tensor_scalar in 4x
            # mode, so phi(Q) costs 2 DVE passes instead of 3.
            nc.scalar.activation(tq[:], qt_t[:], AF.Exp)
            nc.vector.tensor_scalar(qt_t[:], qt_t[:], 0.0, 1.0, ALU.max, ALU.add)
            nc.vector.tensor_tensor(qt_t[:], qt_t[:], tq[:], ALU.min)

            # block-diag C (numer cols only) and ksum, in fp16. The
            # off-diagonal zeros survive pool-buffer reuse (only the diag
            # blocks are rewritten), so memset just the first 2 batches,
            # on the otherwise-idle GpSimd engine.
            c_sb = c_pool.tile([P, P], DT)
            ks_sb = ks_pool.tile([P, HPC], DT)
            if b < 2:
                nc.gpsimd.memset(c_sb[:], 0.0)
                nc.gpsimd.memset(ks_sb[:], 0.0)
            nc.vector.tensor_copy(c_sb[0:E, 0:E], psc[0:E, 0:E])
            nc.vector.tensor_copy(c_sb[E:P, E:P], psc[E:P, EA : EA + E])
            nc.vector.tensor_copy(ks_sb[0:E, 0:1], psc[0:E, E : E + 1])
            nc.vector.tensor_copy(ks_sb[E:P, 1:2], psc[E:P, EA + E : W2])

            # denominators for all 32 chunks into one psum bank
            psd = psd_pool.tile([P, 512], F32)  # full psum bank
            r_sb = r_pool.tile([P, NT * HPC], F32)

            def emit_mmd(lo_n, hi_n):
                for n in range(lo_n, hi_n):
                    nc.tensor.matmul(
                        psd[:, n * HPC : (n + 1) * HPC],
                        lhsT=qt_t[:, n * P : (n + 1) * P],
                        rhs=ks_sb[:],
                        start=True,
                        stop=True,
                    )
                nc.vector.reciprocal_approx_fast(
                    r_sb[:, lo_n * HPC : hi_n * HPC],
                    psd[:, lo_n * HPC : hi_n * HPC],
                )

            if b < B - 1:
                emit_mmd(0, NT)  # byte-identical to the verified emission
            else:
                emit_mmd(0, NT // 2)

            # numerators + normalize, 8 chunks per 2-bank psum tile; the
            # evac multiply reads psum fp32 (1x mode) - batch 1024 dense
            # cols per instruction
            out_sb = out_pool.tile([P, T], DT)
            for g in range(NT // GRP):
                if b == B - 1 and g == 2:
                    # last batch only (nothing downstream): second mm_d
                    # half + its recipf slot between evac groups so the
                    # evac chain starts right after mm_d's first half
                    emit_mmd(NT // 2, NT)
                pso = pso_pool.tile([P, GRP * P], F32)  # 2 psum banks
                for j in range(GRP):
                    n = g * GRP + j
                    nc.tensor.matmul(
                        pso[:, j * P : (j + 1) * P],
                        lhsT=qt_t[:, n * P : (n + 1) * P],
                        rhs=c_sb[:],
                        start=True,
                        stop=True,
                    )
                gl = g * GRP * P
                spl = 2 if (b == B - 1 and g == NT // GRP - 1) else 1
                w = GRP // spl
                for s in range(spl):
                    ov = out_sb[:, gl + s * w * P : gl + (s + 1) * w * P].rearrange(
                        "p (n h e) -> p n h e", n=w, h=HPC
                    )
                    iv = pso[:, s * w * P : (s + 1) * w * P].rearrange(
                        "p (n h e) -> p n h e", n=w, h=HPC
                    )
                    rl = g * GRP * HPC + s * w * HPC
                    rv = (
                        r_sb[:, rl : rl + w * HPC]
                        .rearrange("p (n h) -> p n h", h=HPC)
                        .unsqueeze(3)
                        .broadcast_to((P, w, HPC, E))
                    )
                    nc.vector.tensor_tensor(ov, iv, rv, ALU.mult)
                    if b == B - 1:
                        # last batch: stream each evac group out so the
                        # final DMA only waits on the last 512 cols
                        nc.sync.dma_start(
                            o[b, :, gl + s * w * P : gl + (s + 1) * w * P],
                            out_sb[:, gl + s * w * P : gl + (s + 1) * w * P],
                        )
                if b < B - 1 and g == 1:
                    nc.sync.dma_start(o[b, :, 0 : T // 2], out_sb[:, 0 : T // 2])
            if b < B - 1:
                nc.sync.dma_start(o[b, :, T // 2 : T], out_sb[:, T // 2 : T])
    nc.finalize()
    return nc


_NC_CACHE = None


def _get_nc():
    global _NC_CACHE
    if _NC_CACHE is None:
        _NC_CACHE = build_nc()
    return _NC_CACHE


def make_in_maps(query, key, value):
    query = np.ascontiguousarray(query, dtype=np.float32)
    key = np.ascontiguousarray(key, dtype=np.float32)
    value = np.ascontiguousarray(value, dtype=np.float32)
    in_maps = []
    for c in range(NCORES):
        lo = c * P
        hi = lo + P
        # qt: col n*128+j <-> t = j*32+n
        qt = query[:, :, lo:hi].transpose(0, 2, 1)  # (B, 128, T) t-major
        qt = np.ascontiguousarray(
            qt.reshape(B, P, P, NT).transpose(0, 1, 3, 2)
        ).reshape(B, P, T)
        # k region: (B, p, n, h, e); t = p*32+n
        kk = key[:, :, lo:hi].reshape(B, P, NT, HPC, E)
        # va region: ones col appended per head, cols (n, h, m)
        va = np.empty((B, P, NT, HPC, EA), np.float32)
        va[..., :E] = value[:, :, lo:hi].reshape(B, P, NT, HPC, E)
        va[..., E] = 1.0
        kvb = np.concatenate(
            [kk.reshape(B, P, KC), va.reshape(B, P, VC)], axis=2
        )
        in_maps.append(
            {"qt": qt.astype(F16), "kv": np.ascontiguousarray(kvb).astype(F16)}
        )
    return in_maps


def assemble_out(results):
    out = np.empty((B, T, D), np.float32)
    for c in range(NCORES):
        oc = np.asarray(results[c]["o"], dtype=np.float32)  # (B, 128, 4096)
        # col = n*128 + h*64 + e; partition p <-> t = p*32+n
        out[:, :, c * P : (c + 1) * P] = oc.reshape(B, T, P)
    return out


def run(query, key, value, **spmd_kwargs):
    nc = _get_nc()
    in_maps = make_in_maps(query, key, value)
    res = run_bass_kernel_spmd(nc, in_maps, core_ids=list(range(NCORES)), **spmd_kwargs)
    return assemble_out(res.results), res


def kernel(query, key, value):
    out, _ = run(query, key, value)
    return out


# revision 39
# speedup vs baseline: 1.0198x; 1.0198x over previous
"""Linear attention (non-causal, elu+1 feature map) on 8 Trainium2 cores.

Math per (batch b, head h), with phi(x) = elu(x)+1 = min(exp(x),1) + relu(x):
    C_aug = phi(K)^T @ [V | 1]        # (64, 65): context (64x64) + k_sum col
    numer = phi(Q) @ C                # (T, 64)
    denom = phi(Q) @ k_sum            # (T,)
    out   = numer / denom             # eps=1e-6 negligible vs denom ~1e5

Sharding: 16 heads / 8 cores = 2 heads per core, all 4 batches per core.
Everything on device is fp16 (halves HBM traffic; rel err ~1.3e-2 < 2e-2).

Both heads are fused into single 128-wide matmuls:
  mm1: lhsT = [phiK0 | phiK1] (128t x 128), rhs = [VA0 | VA1] (128t x 130)
       -> psum (128 x 130); diagonal 64x65 blocks are C_aug per head,
       accumulated over 32 t-tiles as TWO matmuls per tile (min-part from
       tk, relu-part from kv in place) so phi-K's add never runs (C is
       linear in phiK).
  mm_d: lhsT = phiQ chunk (128e x 128t), rhs = blockdiag ksum (128 x 2)
       -> denom psum (128t x 2) per chunk, all 32 chunks in one bank so
       a single reciprocal_approx_fast covers them.
  mm2: same lhsT, rhs = blockdiag C (128 x 128) -> numer psum; 8 chunks
       fill a 2-bank psum tile (1024 f32), so normalize+evac is one dense
       1024-col DVE multiply per group (psum fp32 forces 1x mode).

All DMA rides the Sync HW descriptor ring (the GpSimd ring measured
consistently slower end-to-end).  phi(K) runs in 1024-col chunks
chasing the kv DMA so mm1 starts early; phi(Q) overlaps mm1.  The last
batch streams each evac group out immediately, final group split in two
512-col pieces so the closing DMA waits on almost nothing.

Engine notes (all measured on HW): GpSimd bulk elementwise is ~20x
slower than DVE and starves DVE via shared SBUF ports - only memsets go
there.  scalar_tensor_tensor has no 2x mode (7.8us/pass), but the
dual-op tensor_scalar DOES run 4x: phi(Q) uses the identity
elu(q)+1 == min(exp q, relu(q)+1) so relu+1 is one 4x pass and the
min one 2x tensor_tensor - two DVE passes instead of three (-4.3us
measured off the pacing engine).  The same identity loses on the K
side: materializing phi(K) adds a 2x min pass per chunk to DVE,
whereas the 2-matmul PE fold keeps DVE at one 4x pass.  The emission order below is empirically
phase-tuned: the ~14us batch period sits near the device's ~13.7us
full/half-speed throttle duty cycle, so seemingly-neutral reorderings
(coarser phi-K chunks, split qt DMA, skewed cross-batch emission)
measured 5-15us slower; do not perturb without re-measuring several
times - run-to-run variance on identical code spans ~80-95us depending
on device thermal state.

Device layouts (per core, all fp16, partition dim first, all APs dense):
    qt: (B, 128, 4096)  qt[b, hh*64+e, n*128+j] = Q[b, t=j*32+n, ch]
    kv: (B, 128, 8256)  cols 0:4096   = K  [n, h, e] (n*128+h*64+e)
                        cols 4096:8256= VA [n, h, m] (n*130+h*65+m, m=64 ones)
                        partition p <-> t = p*32+n
    o:  (B, 128, 4096)  o[b, p, n*128+h*64+e] = out[b, t=p*32+n, h*64+e]
The t = p*32+n tiling gives every DMA 4-8 KB contiguous per partition.
"""

from contextlib import ExitStack

import numpy as np

import concourse.bacc as bacc
import concourse.bass as bass
import concourse.mybir as mybir
import concourse.tile as tile
from concourse.bass_utils import run_bass_kernel_spmd

B = 4
T = 4096
D = 1024
H = 16
E = 64
EA = E + 1
W2 = 2 * EA  # 130 cols: both heads' [VA]
NCORES = 8
HPC = H // NCORES  # 2 heads per core
P = 128
NT = T // P  # 32 t-tiles
KC = HPC * NT * E  # 4096 k-region cols
VC = HPC * NT * EA  # 4160 va-region cols
KV = KC + VC  # 8256
GRP = 8  # mm2 chunks per evac group (8*128 fp32 = 4096 B = 2 psum banks)
NCH = 4  # phi-K chunks per batch
DT = mybir.dt.float16
F32 = mybir.dt.float32
AF = mybir.ActivationFunctionType
ALU = mybir.AluOpType
F16 = np.float16


def build_nc():
    nc = bacc.Bacc("TRN2", target_bir_lowering=False, debug=False)
    qt = nc.dram_tensor("qt", [B, P, T], DT, kind="ExternalInput").ap()
    kv = nc.dram_tensor("kv", [B, P, KV], DT, kind="ExternalInput").ap()
    o = nc.dram_tensor("o", [B, P, T], DT, kind="ExternalOutput").ap()

    with tile.TileContext(nc) as tc, ExitStack() as ctx:
        qt_pool = ctx.enter_context(tc.tile_pool(name="qt", bufs=3))
        kv_pool = ctx.enter_context(tc.tile_pool(name="kv", bufs=3))
        tmpk_pool = ctx.enter_context(tc.tile_pool(name="tmpk", bufs=3))
        tmpq_pool = ctx.enter_context(tc.tile_pool(name="tmpq", bufs=2))
        c_pool = ctx.enter_context(tc.tile_pool(name="c", bufs=2))
        ks_pool = ctx.enter_context(tc.tile_pool(name="ks", bufs=2))
        r_pool = ctx.enter_context(tc.tile_pool(name="r", bufs=2))
        out_pool = ctx.enter_context(tc.tile_pool(name="out", bufs=3))
        psc_pool = ctx.enter_context(tc.tile_pool(name="psc", bufs=2, space="PSUM"))
        pso_pool = ctx.enter_context(tc.tile_pool(name="pso", bufs=2, space="PSUM"))
        psd_pool = ctx.enter_context(tc.tile_pool(name="psd", bufs=2, space="PSUM"))

        for b in range(B):
            kv_t = kv_pool.tile([P, KV], DT)
            qt_t = qt_pool.tile([P, T], DT)
            kq = KC // 4
            if b == 0:
                # batch 0 is the pipeline head. Everything rides the Sync
                # HW ring (the GpSimd ring measured consistently slower);
                # ring FIFO = need order: K chunks feed the exp chain, VA
                # feeds mm1's rhs, qt halves feed the interleaved expQ.
                vh = KC + (NT // 2) * W2
                nc.sync.dma_start(kv_t[:, 0:kq], kv[b, :, 0:kq])
                nc.sync.dma_start(kv_t[:, KC:vh], kv[b, :, KC:vh])
                for i in range(1, 4):
                    nc.sync.dma_start(
                        kv_t[:, i * kq : (i + 1) * kq],
                        kv[b, :, i * kq : (i + 1) * kq],
                    )
                nc.sync.dma_start(kv_t[:, vh:KV], kv[b, :, vh:KV])
                nc.sync.dma_start(qt_t[:], qt[b])
            else:
                nc.sync.dma_start(kv_t[:, 0 : KC // 2], kv[b, :, 0 : KC // 2])
                nc.sync.dma_start(kv_t[:, KC // 2 : KC], kv[b, :, KC // 2 : KC])
                nc.sync.dma_start(kv_t[:, KC:KV], kv[b, :, KC:KV])
                nc.sync.dma_start(qt_t[:], qt[b])

            # phi(K) = min(exp K, 1) + relu(K), but the add never happens:
            # mm1 accumulates BOTH parts into psum (C is linear in phiK).
            # Per tile: 2 matmuls (relu part from kv_t, min part from tk).
            psc = psc_pool.tile([P, 512], F32)  # full psum bank
            tq = tmpq_pool.tile([P, T], DT)
            kchunk = KC // NCH
            tpc = NT // NCH
            for c in range(NCH):
                kreg = kv_t[:, c * kchunk : (c + 1) * kchunk]
                tk = tmpk_pool.tile([P, kchunk], DT)
                nc.scalar.activation(tk[:], kreg, AF.Exp)
                nc.vector.tensor_scalar_min(tk[:], tk[:], 1.0)
                if b == 0:
                    # head batch: DVE is idle here and ACT's serial
                    # exp chunk chain gates the first matmuls
                    nc.vector.tensor_scalar_max(kreg, kreg, 0.0)
                else:
                    nc.scalar.activation(kreg, kreg, AF.Relu)
                for n in range(c * tpc, (c + 1) * tpc):
                    va = kv_t[:, KC + n * W2 : KC + (n + 1) * W2]
                    off = (n - c * tpc) * P
                    nc.tensor.matmul(
                        psc[:, 0:W2],
                        lhsT=kv_t[:, n * P : (n + 1) * P],
                        rhs=va,
                        start=(n == 0),
                        stop=False,
                    )
                    nc.tensor.matmul(
                        psc[:, 0:W2],
                        lhsT=tk[:, off : off + P],
                        rhs=va,
                        start=False,
                        stop=(n == NT - 1),
                    )
            # phi(Q) via the identity elu(q)+1 == min(exp q, relu(q)+1)
            # (for q>0, e^q >= q+1 so min picks q+1; for q<=0 relu+1 = 1
            # clamps exp).  relu(q)+1 is ONE dual-op tensor_scalar in 4x
            # mode, so phi(Q) costs 2 DVE passes instead of 3.
            nc.scalar.activation(tq[:], qt_t[:], AF.Exp)
            nc.vector.tensor_scalar(qt_t[:], qt_t[:], 0.0, 1.0, ALU.max, ALU.add)
            nc.vector.tensor_tensor(qt_t[:], qt_t[:], tq[:], ALU.min)

            # block-diag C (numer cols only) and ksum, in fp16. The
            # off-diagonal zeros survive pool-buffer reuse (only the diag
            # blocks are rewritten), so memset just the first 2 batches,
            # on the otherwise-idle GpSimd engine.
            c_sb = c_pool.tile([P, P], DT)
            ks_sb = ks_pool.tile([P, HPC], DT)
            if b < 2:
                nc.gpsimd.memset(c_sb[:], 0.0)
                nc.gpsimd.memset(ks_sb[:], 0.0)
            nc.vector.tensor_copy(c_sb[0:E, 0:E], psc[0:E, 0:E])
            nc.vector.tensor_copy(c_sb[E:P, E:P], psc[E:P, EA : EA + E])
            nc.vector.tensor_copy(ks_sb[0:E, 0:1], psc[0:E, E : E + 1])
            nc.vector.tensor_copy(ks_sb[E:P, 1:2], psc[E:P, EA + E : W2])

            # denominators for all 32 chunks into one psum bank
            psd = psd_pool.tile([P, 512], F32)  # full psum bank
            for n in range(NT):
                nc.tensor.matmul(
                    psd[:, n * HPC : (n + 1) * HPC],
                    lhsT=qt_t[:, n * P : (n + 1) * P],
                    rhs=ks_sb[:],
                    start=True,
                    stop=True,
                )
            r_sb = r_pool.tile([P, NT * HPC], F32)
            nc.vector.reciprocal_approx_fast(r_sb[:], psd[:, 0 : NT * HPC])

            # numerators + normalize, 8 chunks per 2-bank psum tile; the
            # evac multiply reads psum fp32 (1x mode) - batch 1024 dense
            # cols per instruction
            out_sb = out_pool.tile([P, T], DT)
            for g in range(NT // GRP):
                pso = pso_pool.tile([P, GRP * P], F32)  # 2 psum banks
                for j in range(GRP):
                    n = g * GRP + j
                    nc.tensor.matmul(
                        pso[:, j * P : (j + 1) * P],
                        lhsT=qt_t[:, n * P : (n + 1) * P],
                        rhs=c_sb[:],
                        start=True,
                        stop=True,
                    )
                gl = g * GRP * P
                spl = 2 if (b == B - 1 and g == NT // GRP - 1) else 1
                w = GRP // spl
                for s in range(spl):
                    ov = out_sb[:, gl + s * w * P : gl + (s + 1) * w * P].rearrange(
                        "p (n h e) -> p n h e", n=w, h=HPC
                    )
                    iv = pso[:, s * w * P : (s + 1) * w * P].rearrange(
                        "p (n h e) -> p n h e", n=w, h=HPC
                    )
                    rl = g * GRP * HPC + s * w * HPC
                    rv = (
                        r_sb[:, rl : rl + w * HPC]
                        .rearrange("p (n h) -> p n h", h=HPC)
                        .unsqueeze(3)
                        .broadcast_to((P, w, HPC, E))
                    )
                    nc.vector.tensor_tensor(ov, iv, rv, ALU.mult)
                    if b == B - 1:
                        # last batch: stream each evac group out so the
                        # final DMA only waits on the last 512 cols
                        nc.sync.dma_start(
                            o[b, :, gl + s * w * P : gl + (s + 1) * w * P],
                            out_sb[:, gl + s * w * P : gl + (s + 1) * w * P],
                        )
                if b < B - 1 and g == 1:
                    nc.sync.dma_start(o[b, :, 0 : T // 2], out_sb[:, 0 : T // 2])
            if b < B - 1:
                nc.sync.dma_start(o[b, :, T // 2 : T], out_sb[:, T // 2 : T])
    nc.finalize()
    return nc


_NC_CACHE = None


def _get_nc():
    global _NC_CACHE
    if _NC_CACHE is None:
        _NC_CACHE = build_nc()
    return _NC_CACHE


def make_in_maps(query, key, value):
    query = np.ascontiguousarray(query, dtype=np.float32)
    key = np.ascontiguousarray(key, dtype=np.float32)
    value = np.ascontiguousarray(value, dtype=np.float32)
    in_maps = []
    for c in range(NCORES):
        lo = c * P
        hi = lo + P
        # qt: col n*128+j <-> t = j*32+n
        qt = query[:, :, lo:hi].transpose(0, 2, 1)  # (B, 128, T) t-major
        qt = np.ascontiguousarray(
            qt.reshape(B, P, P, NT).transpose(0, 1, 3, 2)
        ).reshape(B, P, T)
        # k region: (B, p, n, h, e); t = p*32+n
        kk = key[:, :, lo:hi].reshape(B, P, NT, HPC, E)
        # va region: ones col appended per head, cols (n, h, m)
        va = np.empty((B, P, NT, HPC, EA), np.float32)
        va[..., :E] = value[:, :, lo:hi].reshape(B, P, NT, HPC, E)
        va[..., E] = 1.0
        kvb = np.concatenate(
            [kk.reshape(B, P, KC), va.reshape(B, P, VC)], axis=2
        )
        in_maps.append(
            {"qt": qt.astype(F16), "kv": np.ascontiguousarray(kvb).astype(F16)}
        )
    return in_maps


def assemble_out(results):
    out = np.empty((B, T, D), np.float32)
    for c in range(NCORES):
        oc = np.asarray(results[c]["o"], dtype=np.float32)  # (B, 128, 4096)
        # col = n*128 + h*64 + e; partition p <-> t = p*32+n
        out[:, :, c * P : (c + 1) * P] = oc.reshape(B, T, P)
    return out


def run(query, key, value, **spmd_kwargs):
    nc = _get_nc()
    in_maps = make_in_maps(query, key, value)
    res = run_bass_kernel_spmd(nc, in_maps, core_ids=list(range(NCORES)), **spmd_kwargs)
    return assemble_out(res.results), res


def kernel(query, key, value):
    out, _ = run(query, key, value)
    return out
